# revision 2
# baseline (speedup 1.0000x reference)
"""BitLinear (int8-activation x ternary-weight) matmul on 8 TRN2 NeuronCores.

Full inputs: x [4, 4096, 2048] f32, weight [2048, 2048] f32.
Output: [4, 4096, 2048] fp16 = ((qx @ qw.T) / si / sw).astype(f16).

v3: data-parallel over rows (2048 rows/core). Findings baked in from
v1/v2 traces:
- A hardware AllReduce of the mean|W| partials costs ~62us on this
  stack (vs 9.7us documented floor) -- worse than the ~54us local mean
  it was meant to hide. Dropped; the mean is computed locally with DVE
  reduces pipelined behind the single full W read (DMA-bound, 400GB/s).
- GpSimd elementwise ops run at ~9 G elem/s (30us per [128,2048] tile)
  and stall concurrent DVE work; GpSimd is not used for tiles at all.
- W is read exactly once: 14 of 16 k-tiles stay cached in SBUF through
  the mean pass; the 2 evicted ones are re-read (2.1MB) which overlaps
  the quantize pipeline of the 14 cached tiles.
- Ternarization is 2 ops/k-tile: DVE round (w*sw + 1.5*2^23, RNE) and
  ACT Sign(u - MAGIC) straight to fp8 (sign(n) == clip(n,-1,1) for
  integer n); 3 of 16 k-tiles take a DVE clip+sub emit instead to
  balance the two engine queues.
- qwT is 16 separate tiles so Tile's whole-tile dep tracking cannot
  serialize the quantize producers against the ramp consumers.
- The first two row tiles run kt-interleaved (8 matmuls per quantized
  k-tile, 2 tiles x 4 PSUM banks); ~165 junk matmuls keep the HAM
  clock gate at 2.4 GHz until the ramp starts (~56us).
- x0/x1 activation scaling runs on the (otherwise idle) Scalar engine
  during the mean pass so the DVE reduce chain is not delayed.
"""

import numpy as np

import concourse.mybir as mybir
import concourse.tile as tile
from concourse import bacc
from concourse.bass import ts
from concourse.bass_utils import run_bass_kernel_spmd

N_CORES = 8
ROWS_TOTAL = 4 * 4096
K = 2048
N = 2048
MAGIC = 12582912.0  # 1.5*2^23: fp32 round-to-nearest-even (both signs)
NCACHE = 13  # W k-tiles kept in SBUF through the mean pass

f32 = mybir.dt.float32
bf16 = mybir.dt.bfloat16
f16 = mybir.dt.float16
fp8 = mybir.dt.float8e4
Alu = mybir.AluOpType
Act = mybir.ActivationFunctionType
AxX = mybir.AxisListType.X

# k-tiles whose fp8 emit runs on DVE (clip+sub) instead of ACT Sign,
# balancing the two queues: DVE 16*1.2+3*2.4us vs ACT 13*2.0us.
DVE_EMIT = {4, 9, 14}


def build(rows_per_core=ROWS_TOTAL // N_CORES):
    nc = bacc.Bacc(
        "TRN2", target_bir_lowering=False, debug=False, num_devices=N_CORES
    )
    x_ext = nc.declare_dram_parameter("x", [rows_per_core, K], f32, isOutput=False)
    wt_ext = nc.declare_dram_parameter("wt", [K, N], f32, isOutput=False)
    out_ext = nc.declare_dram_parameter(
        "out", [rows_per_core, N], f16, isOutput=True
    )

    KT = K // 128
    MT = rows_per_core // 128
    NQ = N // 512

    with tile.TileContext(nc) as tc:
        with (
            tc.tile_pool(name="xin", bufs=2) as xin,  # [128,K] f32 x loads
            tc.tile_pool(name="wch", bufs=NCACHE) as wch,  # cached W tiles
            tc.tile_pool(name="wld", bufs=2) as wld,  # streamed + re-read W
            tc.tile_pool(name="qtmp", bufs=2) as qtmp,  # qx bf16
            tc.tile_pool(name="qxt", bufs=4) as qxtp,  # [128,KT,128] bf16 x^T
            tc.tile_pool(name="outp", bufs=2) as outp,  # [128,N] f16 results
            tc.tile_pool(name="singles", bufs=1) as singles,
            tc.tile_pool(name="small", bufs=8) as small,  # [128,1] stats
            tc.tile_pool(name="pacc", bufs=8, space="PSUM") as pacc,
        ):
            # ---- input DMA issue order (one FIFO queue => arrival order):
            # W tiles 0..15 | x 0,1 | W re-read 14,15 | x 2..5 (later)
            wtiles = {}
            for kt in range(KT):
                pool, tag = (wch, "wch") if kt < NCACHE else (wld, "wld")
                wt_t = pool.tile([128, K], f32, tag=tag, name=f"w{kt}")
                nc.sync.dma_start(out=wt_t, in_=wt_ext[ts(kt, 128), :])
                wtiles[kt] = wt_t
            x_pre = {}
            for mi in range(2):
                x_t = xin.tile([128, K], f32, tag="xin", name=f"xpre{mi}")
                nc.sync.dma_start(out=x_t, in_=x_ext[ts(mi, 128), :])
                x_pre[mi] = x_t

            ones_mat = singles.tile([128, 128], f32)
            nc.vector.memset(ones_mat, 1.0)
            negmagic = singles.tile([128, 1], f32)
            nc.vector.memset(negmagic, -MAGIC)
            qwT = [
                singles.tile([128, N], fp8, name=f"qwT{kt}") for kt in range(KT)
            ]

            # ---- PE warm-up junk matmuls: hold the HAM clock gate at
            # 2.4 GHz until the real ramp starts (~56us in)
            warm_src = singles.tile([128, 512], bf16)
            nc.vector.memset(warm_src, 1.0)
            for wi in range(165):
                pwarm = pacc.tile([128, 512], f32, tag="acc", name=f"warm{wi}")
                nc.tensor.matmul(
                    pwarm, lhsT=warm_src[:, :128], rhs=warm_src,
                    start=True, stop=True, skip_group_check=True,
                )

            # ---- mean|W| pass: DVE reduces pipelined behind the W DMA.
            # x0/x1 amax reduces are interleaved into the DVE queue at the
            # points where their tiles have arrived; their scale/emit ops
            # run on the idle Scalar engine.
            wsums = singles.tile([128, KT], f32)

            def w_reduce(kt):
                nc.vector.tensor_reduce(
                    out=wsums[:, kt : kt + 1],
                    in_=wtiles[kt],
                    axis=AxX,
                    op=Alu.add,
                    apply_absolute_value=True,
                )

            def x_quant_head(mi):
                # amax chain on DVE (cheap ops), scaling on ACT
                x_t = x_pre[mi]
                amax = small.tile([128, 1], f32, tag="small")
                nc.vector.tensor_reduce(
                    out=amax, in_=x_t, axis=AxX, op=Alu.max,
                    apply_absolute_value=True,
                )
                amc = small.tile([128, 1], f32, tag="amc", name=f"amc{mi}")
                nc.vector.tensor_scalar_max(out=amc, in0=amax, scalar1=1e-5)
                rec = small.tile([128, 1], f32, tag="small")
                nc.vector.reciprocal(out=rec, in_=amc)
                si = small.tile([128, 1], f32, tag="small")
                nc.vector.tensor_scalar_mul(out=si, in0=rec, scalar1=127.0)
                nc.scalar.activation(
                    out=x_t, in_=x_t, func=Act.Copy, scale=si, bias=MAGIC
                )
                qx = qtmp.tile([128, K], bf16, tag="qtmp")
                nc.scalar.activation(
                    out=qx, in_=x_t, func=Act.Copy, bias=-MAGIC
                )
                qxT = qxtp.tile(
                    [128, KT, 128], bf16, tag="qxt", name=f"qxT{mi}"
                )
                nc.sync.dma_start_transpose(out=qxT, in_=qx)
                return qxT, amc

            for kt in range(6):
                w_reduce(kt)
            xq0 = x_quant_head(0)
            for kt in range(6, 9):
                w_reduce(kt)
            xq1 = x_quant_head(1)
            for kt in range(9, KT):
                w_reduce(kt)

            wtot = small.tile([128, 1], f32, tag="small")
            nc.vector.tensor_reduce(out=wtot, in_=wsums, axis=AxX, op=Alu.add)
            # ones_mat.T @ wtot replicates the grand total across all 128
            # partitions in one matmul
            ptot_b = pacc.tile([128, 1], f32, tag="acc", name="ptot_b")
            nc.tensor.matmul(ptot_b, lhsT=ones_mat, rhs=wtot, start=True, stop=True)
            meanc_b = small.tile([128, 1], f32, tag="s1")
            nc.vector.tensor_scalar(
                out=meanc_b,
                in0=ptot_b,
                scalar1=1.0 / (K * N),
                scalar2=1e-5,
                op0=Alu.mult,
                op1=Alu.max,
            )
            sw_b = singles.tile([128, 1], f32)
            nc.vector.reciprocal(out=sw_b, in_=meanc_b)
            q_b = singles.tile([128, 1], f32)
            nc.vector.tensor_scalar_mul(out=q_b, in0=meanc_b, scalar1=1.0 / 127.0)

            # re-read the two streamed-out W tiles (queued behind x0/x1;
            # they arrive long before the quantize pipeline reaches them)
            for kt in range(NCACHE, KT):
                wt_t = wld.tile([128, K], f32, tag="wld", name=f"wr{kt}")
                nc.sync.dma_start(out=wt_t, in_=wt_ext[ts(kt, 128), :])
                wtiles[kt] = wt_t

            # ---- ternarization: DVE round + (ACT Sign | DVE clip+sub)
            for kt in range(KT):
                wt_t = wtiles[kt]
                nc.vector.tensor_scalar(
                    out=wt_t, in0=wt_t, scalar1=sw_b, scalar2=MAGIC,
                    op0=Alu.mult, op1=Alu.add,
                )
                if kt in DVE_EMIT:
                    nc.vector.tensor_scalar(
                        out=wt_t, in0=wt_t,
                        scalar1=MAGIC - 1.0, scalar2=MAGIC + 1.0,
                        op0=Alu.max, op1=Alu.min,
                    )
                    nc.vector.tensor_scalar_add(
                        out=qwT[kt], in0=wt_t, scalar1=-MAGIC
                    )
                else:
                    nc.scalar.activation(
                        out=qwT[kt], in_=wt_t, func=Act.Sign, bias=negmagic
                    )

            # ---- steady-state x quantization (DVE + one ACT emit)
            def x_quant(mi):
                x_t = xin.tile([128, K], f32, tag="xin", name=f"x{mi}")
                nc.sync.dma_start(out=x_t, in_=x_ext[ts(mi, 128), :])
                amax = small.tile([128, 1], f32, tag="small")
                nc.vector.tensor_reduce(
                    out=amax, in_=x_t, axis=AxX, op=Alu.max,
                    apply_absolute_value=True,
                )
                amc = small.tile([128, 1], f32, tag="amc", name=f"amc{mi}")
                nc.vector.tensor_scalar_max(out=amc, in0=amax, scalar1=1e-5)
                rec = small.tile([128, 1], f32, tag="small")
                nc.vector.reciprocal(out=rec, in_=amc)
                si = small.tile([128, 1], f32, tag="small")
                nc.vector.tensor_scalar_mul(out=si, in0=rec, scalar1=127.0)
                nc.vector.tensor_scalar(
                    out=x_t, in0=x_t, scalar1=si, scalar2=MAGIC,
                    op0=Alu.mult, op1=Alu.add,
                )
                qx = qtmp.tile([128, K], bf16, tag="qtmp")
                nc.scalar.activation(
                    out=qx, in_=x_t, func=Act.Copy, bias=-MAGIC
                )
                qxT = qxtp.tile(
                    [128, KT, 128], bf16, tag="qxt", name=f"qxT{mi}"
                )
                nc.sync.dma_start_transpose(out=qxT, in_=qx)
                return qxT, amc

            # ---- main loop over row tiles
            def mm(acc, qxT, kt, nq, start, stop):
                nc.tensor.matmul(
                    acc, lhsT=qxT[:, kt, :], rhs=qwT[kt][:, ts(nq, 512)],
                    start=start, stop=stop,
                    skip_group_check=True,
                )

            def finish(mi, accs, amc):
                cs = small.tile([128, 1], f32, tag="small")
                nc.vector.tensor_mul(cs, amc, q_b)  # (amax/127)*meanc
                o_t = outp.tile([128, N], f16, tag="outp", name=f"o{mi}")
                for nq in range(NQ):
                    nc.scalar.activation(
                        out=o_t[:, ts(nq, 512)], in_=accs[nq],
                        func=Act.Copy, scale=cs,
                    )
                nc.scalar.dma_start(out=out_ext[ts(mi, 128), :], in_=o_t)

            # ramp: first two row tiles interleaved across kt so each
            # quantized k-tile unlocks 8 matmuls while quantization runs
            qxT0, amc0 = xq0
            qxT1, amc1 = xq1
            accs0 = [
                pacc.tile([128, 512], f32, tag="acc", name=f"acc_0_{i}")
                for i in range(NQ)
            ]
            accs1 = [
                pacc.tile([128, 512], f32, tag="acc", name=f"acc_1_{i}")
                for i in range(NQ)
            ]
            for kt in range(KT):
                st, sp = kt == 0, kt == KT - 1
                for nq in range(NQ):
                    mm(accs0[nq], qxT0, kt, nq, st, sp)
                for nq in range(NQ):
                    mm(accs1[nq], qxT1, kt, nq, st, sp)
            finish(0, accs0, amc0)
            finish(1, accs1, amc1)

            for mi in range(2, MT):
                qxT, amc = x_quant(mi)
                accs = [
                    pacc.tile([128, 512], f32, tag="acc", name=f"acc_{mi}_{i}")
                    for i in range(NQ)
                ]
                if mi == MT - 1:
                    # nq-inner: each output chunk completes as soon as its
                    # 16 accumulations are done, so the dequant + store
                    # overlap the remaining matmuls (shorter kernel tail)
                    for nq in range(NQ):
                        for kt in range(KT):
                            mm(accs[nq], qxT, kt, nq, kt == 0, kt == KT - 1)
                else:
                    for kt in range(KT):
                        for nq in range(NQ):
                            mm(accs[nq], qxT, kt, nq, kt == 0, kt == KT - 1)
                finish(mi, accs, amc)

    nc.compile()
    return nc


_NC_CACHE = {}


def _get_nc(rows_per_core):
    if rows_per_core not in _NC_CACHE:
        _NC_CACHE[rows_per_core] = build(rows_per_core)
    return _NC_CACHE[rows_per_core]


def run(x, weight, **spmd_kwargs):
    x = np.ascontiguousarray(np.asarray(x, dtype=np.float32))
    weight = np.asarray(weight, dtype=np.float32)
    b, s, k = x.shape
    rows = b * s
    rpc = rows // N_CORES
    xr = x.reshape(rows, k)
    wt = np.ascontiguousarray(weight.T)
    nc = _get_nc(rpc)
    in_maps = [
        {"x": xr[i * rpc : (i + 1) * rpc], "wt": wt} for i in range(N_CORES)
    ]
    res = run_bass_kernel_spmd(
        nc, in_maps, core_ids=list(range(N_CORES)), **spmd_kwargs
    )
    out = np.concatenate(
        [res.results[i]["out"] for i in range(N_CORES)], axis=0
    )
    return out.reshape(b, s, N), res


def kernel(x, weight):
    out, _ = run(x, weight)
    return out
